# revision 1
# baseline (speedup 1.0000x reference)
"""ABMIL attention-pooling kernel for 8 TRN2 NeuronCores (Bass/Tile).

Reference computation (per bag b of B=4, N=20000 instances, 1024 feats):
    h   = x @ W_pe + b_pe                    [N, 512]
    A_V = tanh(h @ W_V + b_V)                [N, 128]
    A_U = sigmoid(h @ W_U + b_U)             [N, 128]
    a   = (A_V * A_U) @ w_att + b_att        [N, 1]
    A   = softmax(a, axis=0)
    M   = sum(A * h, axis=0)                 [512]
    out = M @ W_cls + b_cls                  [2]

Algebraic rewrites used:
  * pooling commutes with the classifier: the kernel only produces the
    unnormalized pooled embedding wh = sum_n w_n h_n and s = sum_n w_n
    (w_n = exp(a_n); softmax shift skipped, logits are O(1)).  The host
    finishes with logits = (wh/s) @ W_cls + b_cls.  This removes the
    per-instance classifier matmuls from the PE entirely.
  * sigmoid(y) = (1 + tanh(y/2))/2 so tanh/exp/identity are the only ACT
    functions -> one ACT table set (exp_and_others), no ~2.7us table swaps.
    The 1/2 factors fold into w_att (host) and the U-branch bias.
  * w_att is replicated to 128 columns host-side so the logit matmul
    broadcasts the logit onto all 128 partitions; the pooled sum
    wh_m = sum_n w_n h16[:, m, n] is then a GPSIMD multiply (stride-0
    broadcast of w over the emb-chunk dim) + DVE free-dim reduce.

Sharding: core c -> bag c//2, instance half c%2 (10000 instances each);
the host sums the two partials per bag and applies the classifier.

Device dataflow is fully "transposed" (embedding dim on partitions) so the
x tile loaded as [feat, inst] feeds every matmul with zero transposes:
    hT  [512, n]  = W_pe.T @ xT       (lhsT = W_pe chunks)
    AVT [128, n]  = W_V.T @ hT         etc.
    lg  [128, n]  = watt128.T @ G      (0.5*w_att replicated to 128 rows =
                                        free broadcast over all partitions)
    w16 [128, n]  = exp(lg + b_att)    (ACT)
    wh, s         = free-dim reductions of w16*h16 and w16 (GPSIMD+DVE)

Compute dtype is float16 (same TensorE rate as bf16, ~5x less quantization
error; all values here are O(1) so fp16 range is safe).  PSUM accumulation
and reductions stay float32.  fp8 (DoubleRow) was measured on this
hardware at the SAME per-instruction cost as f16 (2x flops/instr), so the
3-pass residual-compensated fp8 patch-embed (48 instrs vs 32) loses.

Pipeline: the back phase is split so the PE never waits on the
tanh->gate->logit ACT/DVE chain.  Iteration t emits:
    front(t)   — x DMA + patch-embed matmuls + h16 copies
    back_a(t-1)— A_V/A_U matmuls + tanh + gate product (issued to ACT/DVE)
    back_b(t-2)— logit matmul (g16 ready an embed-block ago), exp,
                 pooled-sum mult/reduce
A PE spin after the loop keeps the HAM clock at 2.4 GHz while the last
tiles' exp/pool chain drains on ACT/GPSIMD/DVE (the clock otherwise halves
within ~1us of PE idleness, doubling the tail).
"""

import os
import sys

import numpy as np

# Shapes for this problem (hardcoded per the task contract).
B = 4
N = 20000
IN_DIM = 1024
EMB = 512
ATT = 128
NCLS = 2
N_CORES = 8
N_SHARD = (B * N) // N_CORES  # 10000 instances per core
TILE = 500                    # instances per device tile
N_TILES = N_SHARD // TILE     # 20
KC = IN_DIM // 128            # 8 feature chunks
MC = EMB // 128               # 4 embedding chunks

# packed weight layout (f16, [128, WPACK_COLS]):
#   [0:4096)        W_pe     as [ki, ko*512+e]
#   [4096:4608)     W_V      as [mi, mo*128+a]
#   [4608:5120)     W_U      as [mi, mo*128+a]
#   [5120:5248)     watt128  (0.5*w_att replicated to 128 cols)
W_PE_OFF = 0
W_V_OFF = 4096
W_U_OFF = 4608
W_ATT_OFF = 5120
WPACK_COLS = 5248

# packed bias layout (f32, [128, 7]):
#   [0:4) b_pe chunks; [4] b_V; [5] 0.5*b_U; [6] b_att (broadcast all rows)
BPACK_COLS = 7

# out layout (f32, [128, 100]): cols [0:80) = wh per (m, tile); on
# partition 0, cols [80:100) = s per tile.
OUT_COLS = MC * N_TILES + N_TILES

_cache = {}


def _import_concourse():
    for p in ("/opt/trn_rl_repo", "/root/.axon_site",
              "/root/.axon_site/_ro/trn_rl_repo"):
        if os.path.isdir(p) and p not in sys.path:
            sys.path.append(p)
    import concourse.bass as bass          # noqa: F401
    import concourse.tile as tile          # noqa: F401
    from concourse import mybir            # noqa: F401
    return bass, tile, mybir


def _build_graph():
    bass, tile, mybir = _import_concourse()
    from concourse import bacc
    from concourse.bass import broadcast_tensor_aps
    f16 = mybir.dt.float16
    f32 = mybir.dt.float32
    AF = mybir.ActivationFunctionType
    ALU = mybir.AluOpType

    nc = bacc.Bacc("TRN2", target_bir_lowering=False, debug=False,
                   num_devices=N_CORES)

    # x ships tile-major: [tile, ki, ko*TILE+inst] so each tile is one DMA
    # with an 8KB contiguous run per partition (4x fewer descriptors and one
    # Sync issue instead of four).
    xT = nc.declare_dram_parameter("xT", [N_TILES, 128, KC * TILE], f16,
                                   isOutput=False)
    wpack = nc.declare_dram_parameter("wpack", [128, WPACK_COLS], f16,
                                      isOutput=False)
    bpack = nc.declare_dram_parameter("bpack", [128, BPACK_COLS], f32,
                                      isOutput=False)
    # per-tile partials go out raw (wh[128, 4, 20] + s[1, 20]); the
    # 20-column sums and the classifier happen on the host, keeping the
    # kernel tail to two DMAs.
    out = nc.declare_dram_parameter("out", [128, OUT_COLS], f32,
                                    isOutput=True)

    xT_r = xT.rearrange("t p (ko i) -> t p ko i", ko=KC)

    with tile.TileContext(nc) as tc:
        with (
            tc.tile_pool(name="singles", bufs=1) as singles,
            tc.tile_pool(name="xin", bufs=4) as xin,
            tc.tile_pool(name="h16p", bufs=3) as h16p,
            tc.tile_pool(name="gates", bufs=2) as gates,
            tc.tile_pool(name="wexp", bufs=2) as wexp,
            tc.tile_pool(name="acc", bufs=1) as accp,
            tc.tile_pool(name="ps_h", bufs=2, space="PSUM") as ps_h,
            tc.tile_pool(name="ps_av", bufs=1, space="PSUM") as ps_av,
            tc.tile_pool(name="ps_au", bufs=1, space="PSUM") as ps_au,
            tc.tile_pool(name="ps_lg", bufs=1, space="PSUM") as ps_lg,
            tc.tile_pool(name="ps_warm", bufs=1, space="PSUM") as ps_warm,
        ):
            # PE warm-up spin, emitted FIRST so it runs from ~0.6us: the
            # HAM clock gate keeps the PE at 1.2 GHz until ~3.4us of
            # sustained activity, so a dependency-free matmul spin flips it
            # to 2.4 GHz; the first real matmuls (whose x/weight DMAs land
            # ~3us in with the interleaved DMA ladder below) then keep it
            # warm.  Sized to end right as those DMAs land.
            warm_sb = singles.tile([128, 512], f16)
            nc.vector.memset(warm_sb, 0.0)
            warm_ps = ps_warm.tile([128, 512], f32, tag="warm")
            # free=512 spins (~210ns each, not pipelineable away) bridge the
            # PE from t~0.3us until the first embed deps land (~6.5us).
            for _ in range(30):
                nc.tensor.matmul(warm_ps[:64, :], lhsT=warm_sb[:, 0:64],
                                 rhs=warm_sb, start=True, stop=True)

            # ---- preamble: interleaved weight/x DMA ladder so the first
            # embed matmul is gated on ~384KB, not ~2MB.  DMA issues cost
            # ~640ns each on the Sync queue, so the first-matmul
            # dependencies (W_pe k0, x k0-1) go first.
            wp = singles.tile([128, WPACK_COLS], f16)
            xt0 = xin.tile([128, KC, TILE], f16, tag="xt")
            nc.sync.dma_start(out=wp[:, 0:512], in_=wpack[:, 0:512])
            nc.sync.dma_start(out=xt0[:, 0:2, :], in_=xT_r[0, :, 0:2, :])
            nc.sync.dma_start(out=wp[:, 512:1536], in_=wpack[:, 512:1536])
            nc.sync.dma_start(out=xt0[:, 2:4, :], in_=xT_r[0, :, 2:4, :])
            nc.sync.dma_start(out=wp[:, 1536:W_V_OFF],
                              in_=wpack[:, 1536:W_V_OFF])
            nc.sync.dma_start(out=xt0[:, 4:8, :], in_=xT_r[0, :, 4:8, :])
            nc.sync.dma_start(out=wp[:, W_V_OFF:], in_=wpack[:, W_V_OFF:])
            bias_sb = singles.tile([128, BPACK_COLS], f32)
            nc.sync.dma_start(out=bias_sb, in_=bpack[:, :])

            wpe = wp[:, W_PE_OFF:W_V_OFF].rearrange("p (ko e) -> p ko e", ko=KC)
            wv = wp[:, W_V_OFF:W_U_OFF].rearrange("p (mo a) -> p mo a", mo=MC)
            wu = wp[:, W_U_OFF:W_ATT_OFF].rearrange("p (mo a) -> p mo a", mo=MC)
            watt128 = wp[:, W_ATT_OFF:WPACK_COLS]

            # DVE-local biases (feed tensor_scalar_add)
            bpe_dve = singles.tile([128, MC], f32)
            nc.vector.tensor_copy(bpe_dve, bias_sb[:, 0:MC])
            # ACT-local biases (feed activation bias port)
            bpe_act = singles.tile([128, MC], f32)
            nc.scalar.activation(bpe_act, bias_sb[:, 0:MC], AF.Identity)
            bvu_act = singles.tile([128, 2], f32)
            nc.scalar.activation(bvu_act, bias_sb[:, 4:6], AF.Identity)
            batt_act = singles.tile([128, 1], f32)
            nc.scalar.activation(batt_act, bias_sb[:, 6:7], AF.Identity)

            acc_s = accp.tile([1, N_TILES], f32)
            acc_wh = accp.tile([128, MC, N_TILES], f32)

            def emit_front(t):
                """DMA + patch-embed matmuls + h16 copies for tile t."""
                if t == 0:
                    xt = xt0  # DMA already issued in the preamble
                else:
                    xt = xin.tile([128, KC, TILE], f16, tag="xt")
                    nc.sync.dma_start(out=xt, in_=xT_r[t])

                h16 = h16p.tile([128, MC, TILE], f16, tag="h16")
                for half in range(2):
                    h_ps = ps_h.tile([128, 2, 512], f32, tag="h_ps")
                    # k-outer: the first matmuls only need the first x/weight
                    # chunks, so streaming starts before the tail chunks land
                    for k in range(KC):
                        for m2 in range(2):
                            m = 2 * half + m2
                            nc.tensor.matmul(
                                h_ps[:, m2, :TILE],
                                lhsT=wpe[:, k, m * 128:(m + 1) * 128],
                                rhs=xt[:, k, :],
                                start=(k == 0), stop=(k == KC - 1),
                            )
                    # PSUM f32 -> SBUF f16 with bias add.  Both chunks on
                    # ACT: the DVE carries the pooled-sum work, so keeping
                    # the copies off it shortens the end-of-kernel DVE
                    # drain (ACT drains in parallel).
                    m0 = 2 * half
                    nc.scalar.activation(
                        h16[:, m0, :], h_ps[:, 0, :TILE], AF.Identity,
                        bias=bpe_act[:, m0:m0 + 1])
                    nc.scalar.activation(
                        h16[:, m0 + 1, :], h_ps[:, 1, :TILE], AF.Identity,
                        bias=bpe_act[:, m0 + 1:m0 + 2])
                return h16

            def emit_back_a(t, h16):
                """A_V/A_U matmuls + tanh + gate product for tile t.  The
                gate chain retires on ACT/DVE during the NEXT tile's embed
                block, so back_b's logit matmul never waits on it."""
                av_ps = ps_av.tile([128, 512], f32, tag="av")
                au_ps = ps_au.tile([128, 512], f32, tag="au")
                for m in range(MC):
                    nc.tensor.matmul(
                        av_ps[:, :TILE], lhsT=wv[:, m, :], rhs=h16[:, m, :],
                        start=(m == 0), stop=(m == MC - 1),
                    )
                    nc.tensor.matmul(
                        au_ps[:, :TILE], lhsT=wu[:, m, :], rhs=h16[:, m, :],
                        start=(m == 0), stop=(m == MC - 1),
                    )

                # ACT: gv = tanh(av + b_V); gu = tanh(au/2 + b_U/2)
                gv = gates.tile([128, TILE], f16, tag="gv")
                nc.scalar.activation(gv, av_ps[:, :TILE], AF.Tanh,
                                     bias=bvu_act[:, 0:1])
                gu = gates.tile([128, TILE], f16, tag="gu")
                nc.scalar.activation(gu, au_ps[:, :TILE], AF.Tanh,
                                     bias=bvu_act[:, 1:2], scale=0.5)
                # DVE: G' = (gu + 1) * gv   (= 2 * A_V * A_U).  NOTE: this
                # op on GPSIMD crashes at runtime (JaxRuntimeError in the
                # axon/pjrt path) — keep it on DVE.
                g16 = gates.tile([128, TILE], f16, tag="g16")
                nc.vector.scalar_tensor_tensor(
                    g16, gu, 1.0, gv, op0=ALU.add, op1=ALU.mult)
                return g16

            def emit_back_b(t, h16, g16, lg_ps):
                """Logit matmul, exp weights and pooled partials for tile
                t.  g16 was issued one embed block ago, so the logit matmul
                runs back-to-back with the preceding PE work."""
                nc.tensor.matmul(lg_ps[:, :TILE], lhsT=watt128, rhs=g16,
                                 start=True, stop=True)

                # ACT: w = exp(logit + b_att) broadcast on every partition.
                # NOTE: accum_out (ACT accumulator / tensor_tensor_reduce)
                # raises NRT_EXEC_UNIT_UNRECOVERABLE on this hardware path,
                # so the reductions run as separate DVE instructions.
                w16 = wexp.tile([128, TILE], f16, tag="w16")
                nc.scalar.activation(w16, lg_ps[:, :TILE], AF.Exp,
                                     bias=batt_act)
                nc.vector.reduce_sum(acc_s[0:1, t:t + 1], w16[0:1, :],
                                     axis=mybir.AxisListType.X)

                # pooled partial wh = sum_n w16 * h16: one DVE multiply
                # (all-f16 SBUF contiguous -> 2x/4x DVE mode; w broadcast
                # over the emb-chunk dim with a stride-0 AP) + one DVE
                # free-dim reduce.  GPSIMD was measured at ~2ns/elem for
                # tensor_mul (4x slower than DVE) - do not offload there.
                prod = wexp.tile([128, MC, TILE], f16, tag="prod")
                w_b, h_b = broadcast_tensor_aps(
                    w16.rearrange("p (o i) -> p o i", o=1), h16[:, :, :])
                nc.vector.tensor_mul(prod[:, :, :], w_b, h_b)
                # per-m reduces: the [128,1] f32 destination is free_size-1
                # (scalar-exempt), so the f16 source streams in 2x DVE mode
                # (a single [128,4,500]->[128,4] reduce runs 1x).
                for m in range(MC):
                    nc.vector.reduce_sum(acc_wh[:, m, t:t + 1],
                                         prod[:, m, :],
                                         axis=mybir.AxisListType.X)
                return w16

            # Observer matmuls: 1x1 reads of the newest ACT output (w16 of
            # t-3) and DVE output (h16 of t-1) at each iteration head advance
            # the PE's semaphore vector clock, so group-start matmuls' WAR
            # legs are pre-observed and bacc emits no event-semaphore
            # relays.  Their dst sits inside the logit region: the logit
            # matmul's start=True overwrites it (value discarded) and exp
            # reads the tile, so DCE keeps them.
            hist = []
            g16s = {}
            w16s = {}
            for t in range(N_TILES + 1):
                lg_ps = ps_lg.tile([128, 512], f32, tag="lg")
                if (t - 3) in w16s:
                    w16p = w16s.pop(t - 3)
                    nc.tensor.matmul(lg_ps[0:1, 0:1], lhsT=w16p[0:2, 0:1],
                                     rhs=w16p[0:2, 0:1], start=True, stop=True)
                if 1 <= t and t - 1 < N_TILES and hist[t - 1] is not None:
                    hp = hist[t - 1]
                    nc.tensor.matmul(lg_ps[0:1, 1:2], lhsT=hp[:, 3, 0:1],
                                     rhs=hp[:, 3, 0:1], start=True, stop=True)
                cur = emit_front(t) if t < N_TILES else None
                hist.append(cur)
                if 0 <= t - 1 < N_TILES - 1:
                    g16s[t - 1] = emit_back_a(t - 1, hist[t - 1])
                if t == N_TILES - 1:
                    # last tile's gates issue one iteration early (its av/au
                    # matmuls chase the h16 copies as they land, ~0.3us PE
                    # stall) so the final pool chain starts an embed block
                    # sooner and the end-of-kernel DVE drain shortens.
                    g16s[t] = emit_back_a(t, hist[t])
                if t >= 2:
                    w16s[t - 2] = emit_back_b(t - 2, hist[t - 2],
                                              g16s.pop(t - 2), lg_ps)
                if t == N_TILES:
                    lg2 = ps_lg.tile([128, 512], f32, tag="lg")
                    w16s[t - 1] = emit_back_b(t - 1, hist[t - 1],
                                              g16s.pop(t - 1), lg2)
                if t - 2 == N_TILES - 3:
                    # early writeback of tiles 0..N_TILES-3: hides the bulk
                    # of the output DMA (issue + transfer) under the last
                    # two tiles' compute; only the last two tiles' columns
                    # ride the tail DMA.  Single strided DMA per tensor
                    # (issues cost ~640ns each on the Sync queue).
                    out_mt = out.rearrange("p (m t) -> p m t", m=MC + 1)
                    nc.sync.dma_start(
                        out=out_mt[:, 0:MC, 0:N_TILES - 2],
                        in_=acc_wh[:, :, 0:N_TILES - 2])
                    nc.sync.dma_start(
                        out=out[0:1, MC * N_TILES:OUT_COLS - 2],
                        in_=acc_s[0:1, 0:N_TILES - 2])

            # tail writeback: the last two tiles' partial columns
            out_mt = out.rearrange("p (m t) -> p m t", m=MC + 1)
            nc.sync.dma_start(out=out_mt[:, 0:MC, N_TILES - 2:N_TILES],
                              in_=acc_wh[:, :, N_TILES - 2:N_TILES])
            nc.sync.dma_start(out=out[0:1, OUT_COLS - 2:OUT_COLS],
                              in_=acc_s[0:1, N_TILES - 2:N_TILES])

            # tail spin: dependency-free matmuls hold the HAM clock at
            # 2.4 GHz while the last tiles' exp/pool chain drains on
            # ACT/DVE (~8us) and the TileContext teardown barrier chain
            # (~5-9us of tiny cross-engine hops) completes; without it the
            # clock halves ~3.4us after PE idle and the whole tail doubles.
            # Sized to end just before the teardown does.
            for _ in range(54):
                nc.tensor.matmul(warm_ps[:64, :], lhsT=warm_sb[:, 0:64],
                                 rhs=warm_sb, start=True, stop=True)

    # Runs the bacc passes (move_matmul_waits_to_ldweights,
    # generate_event_semaphores, DCE, ...) that make the BIR satisfy the
    # hardware's one-sync-wait-per-instruction constraint.
    nc.compile()
    return nc


def _prep_in_maps(x, W_pe, b_pe, W_V, b_V, W_U, b_U, w_att, b_att):
    f16 = np.float16
    f32 = np.float32

    wpack = np.empty((128, WPACK_COLS), dtype=f16)
    wpack[:, W_PE_OFF:W_V_OFF] = (
        W_pe.reshape(KC, 128, EMB).transpose(1, 0, 2).reshape(128, KC * EMB))
    wpack[:, W_V_OFF:W_U_OFF] = (
        W_V.reshape(MC, 128, ATT).transpose(1, 0, 2).reshape(128, MC * ATT))
    wpack[:, W_U_OFF:W_ATT_OFF] = (
        W_U.reshape(MC, 128, ATT).transpose(1, 0, 2).reshape(128, MC * ATT))
    # kernel computes G' = 2*A_V*A_U; 0.5*w_att undoes the factor of 2.
    # Replicated to 128 columns so the logit matmul broadcasts the logit
    # onto every output partition (feeds the pooled-sum multiplies).
    wpack[:, W_ATT_OFF:WPACK_COLS] = np.repeat(
        0.5 * w_att.astype(f32), 128, axis=1)

    bpack = np.empty((128, BPACK_COLS), dtype=f32)
    bpack[:, 0:MC] = b_pe.reshape(MC, 128).T
    bpack[:, 4] = b_V
    # kernel computes tanh(0.5*y + bias) for the U branch -> bias = b_U/2
    bpack[:, 5] = 0.5 * b_U
    bpack[:, 6] = b_att[0]

    shared = {"wpack": wpack, "bpack": bpack}
    in_maps = []
    half = N // 2
    for c in range(N_CORES):
        bag, hi = divmod(c, 2)
        xs = x[bag, hi * half:(hi + 1) * half, :]
        xt_tiles = np.ascontiguousarray(
            xs.T.astype(f16).reshape(KC, 128, N_TILES, TILE)
            .transpose(2, 1, 0, 3).reshape(N_TILES, 128, KC * TILE))
        in_maps.append({"xT": xt_tiles, **shared})
    return in_maps


def _run(inputs, trace=False, tmpdir=None):
    _import_concourse()
    from concourse.bass_utils import run_bass_kernel_spmd

    if "nc" not in _cache:
        _cache["nc"] = _build_graph()
    nc = _cache["nc"]

    in_maps = _prep_in_maps(
        inputs["x"], inputs["W_pe"], inputs["b_pe"], inputs["W_V"],
        inputs["b_V"], inputs["W_U"], inputs["b_U"], inputs["w_att"],
        inputs["b_att"])

    res = run_bass_kernel_spmd(
        nc, in_maps, core_ids=list(range(N_CORES)),
        trace=trace, tmpdir=tmpdir)

    W_cls = np.asarray(inputs["W_cls"], dtype=np.float64)
    b_cls = np.asarray(inputs["b_cls"], dtype=np.float64)
    logits = np.zeros((B, NCLS), dtype=np.float32)
    for bag in range(B):
        M = np.zeros(EMB, dtype=np.float64)
        s = 0.0
        for hi in range(2):
            o = res.results[2 * bag + hi]["out"].astype(np.float64)
            wh = o[:, 0:MC * N_TILES].reshape(128, MC, N_TILES).sum(axis=2)
            M += wh.T.reshape(EMB)  # e = m*128 + p
            s += o[0, MC * N_TILES:OUT_COLS].sum()
        logits[bag] = ((M / s) @ W_cls + b_cls).astype(np.float32)
    return logits, res


def kernel(**inputs):
    inputs = {k: np.asarray(v) for k, v in inputs.items()}
    logits, _ = _run(inputs, trace=False)
    return logits



# revision 2
# speedup vs baseline: 1.3113x; 1.3113x over previous
"""ABMIL attention-pooling kernel for 8 TRN2 NeuronCores (Bass/Tile).

Reference computation (per bag b of B=4, N=20000 instances, 1024 feats):
    h   = x @ W_pe + b_pe                    [N, 512]
    A_V = tanh(h @ W_V + b_V)                [N, 128]
    A_U = sigmoid(h @ W_U + b_U)             [N, 128]
    a   = (A_V * A_U) @ w_att + b_att        [N, 1]
    A   = softmax(a, axis=0)
    M   = sum(A * h, axis=0)                 [512]
    out = M @ W_cls + b_cls

Algebraic rewrites (all weight-only precomputation, done host-side):
  * h @ W_V == x @ (W_pe @ W_V): the attention branches never need h.
    With P_V = W_pe@W_V, P_U = W_pe@W_U (both [1024,128]) and
    bias'_V = b_pe@W_V + b_V etc., the gates are rank-256 projections of
    x directly -- the [1024x512] patch embed disappears from the device.
  * logits = (sum_n w_n h_n / s) @ W_cls + b_cls
           = (sum_n w_n (x_n @ P_cls)) / s + (b_pe@W_cls + b_cls),
    with P_cls = W_pe@W_cls [1024, 2] and w_n = exp(a_n), s = sum w_n.
    So the device only needs z = sum_n w_n c_n (c_n = x_n@P_cls, rank 2)
    and s -- never the 512-dim pooled embedding.
  * The z contraction is split across engines to balance the pipeline:
    feature chunks 0..P_CLS_PE-1 go through PE matmuls (c16 = x@P_cls
    then a DVE w-weighted reduce), chunks P_CLS_PE..7 go through a DVE
    weighted x-sum (xbar_k = sum_n w_n x[k,n]; host finishes
    xbar @ P_cls_k).  PE marginal cost per chunk is ~211ns vs DVE
    ~590ns, so the split point balances PE (~4.5us/tile) vs DVE.
  * sigmoid(y) = (1 + tanh(y/2))/2 so tanh/exp/identity are the only ACT
    functions -> one ACT table set, no table swaps.  0.5 factors fold
    into w_att (host) and the U-branch bias.
  * w_att is replicated to 128 columns host-side so the logit matmul
    broadcasts the logit onto all 128 partitions (feeds the stride-0
    broadcast multiply of w over the x feature-chunk dim).

fp8 was evaluated and rejected: quantizing the P_* weights creates an
instance-correlated attention tilt that does NOT average out in the pool
(measured 8.7e-2 rel err vs the 2e-2 gate), and DoubleRow requires both
operands fp8.  f16 everywhere measures 8.8e-4.

Sharding: core c -> bag c//2, instance half c%2 (10000 instances each);
the host sums the two partials per bag and applies the constant.

Pipeline (software-pipelined 3 deep, per python iteration t):
    PE : cls(t-2) 4MM + logit(t-2) + V(t) 8MM + U(t) 8MM
    ACT: gv(t-1), gu(t-1) tanh; w16(t-2) exp; c16(t-2) copy
    DVE: g16(t-1) gate product; prod4(t-2) w*x; 4 reduces; z/s reduces
PE is the bottleneck engine (~4.5us/iter); ACT ~2.3us, DVE ~3.7us.
"""

import os
import sys

import numpy as np

# Shapes for this problem (hardcoded per the task contract).
B = 4
N = 20000
IN_DIM = 1024
EMB = 512
ATT = 128
NCLS = 2
N_CORES = 8
N_SHARD = (B * N) // N_CORES  # 10000 instances per core
TILE = 500                    # instances per device tile
N_TILES = N_SHARD // TILE     # 20
KC = IN_DIM // 128            # 8 feature chunks
P_CLS_PE = 4                  # cls feature chunks done as PE matmuls
V_CHUNKS = KC - P_CLS_PE      # cls feature chunks done as DVE xbar sums

# packed weight layout (f16, [128, WPACK_COLS]):
#   [0:1024)        P_V     as [ki, k*128+a]
#   [1024:2048)     P_U     as [ki, k*128+a]
#   [2048:2176)     watt128 (0.5*w_att replicated to 128 cols)
#   [2176:2184)     P_cls chunks 0..P_CLS_PE-1 as [ki, k*2+c]
W_V_OFF = 0
W_U_OFF = 1024
W_ATT_OFF = 2048
W_CLS_OFF = 2176
WPACK_COLS = W_CLS_OFF + P_CLS_PE * NCLS

# packed bias layout (f32, [128, 3]): [0] bias'_V; [1] 0.5*bias'_U;
# [2] b_att (broadcast all rows)
BPACK_COLS = 3

# out layout (f32, [128, N_TILES, 6]): [:, t, 0:4] = xbar chunks
# P_CLS_PE..7; [0:2, t, 4] = z partial; [0:1, t, 5] = s partial.
OUT_SEC = V_CHUNKS + 2
OUT_COLS = N_TILES * OUT_SEC

_cache = {}


def _import_concourse():
    for p in ("/opt/trn_rl_repo", "/root/.axon_site",
              "/root/.axon_site/_ro/trn_rl_repo"):
        if os.path.isdir(p) and p not in sys.path:
            sys.path.append(p)
    import concourse.bass as bass          # noqa: F401
    import concourse.tile as tile          # noqa: F401
    from concourse import mybir            # noqa: F401
    return bass, tile, mybir


def _build_graph():
    bass, tile, mybir = _import_concourse()
    from concourse import bacc
    from concourse.bass import broadcast_tensor_aps
    f16 = mybir.dt.float16
    f32 = mybir.dt.float32
    AF = mybir.ActivationFunctionType
    ALU = mybir.AluOpType

    nc = bacc.Bacc("TRN2", target_bir_lowering=False, debug=False,
                   num_devices=N_CORES)

    # x ships tile-major: [tile, ki, k*TILE+inst] so each tile is one DMA
    # with an 8KB contiguous run per partition.
    xT = nc.declare_dram_parameter("xT", [N_TILES, 128, KC * TILE], f16,
                                   isOutput=False)
    wpack = nc.declare_dram_parameter("wpack", [128, WPACK_COLS], f16,
                                      isOutput=False)
    bpack = nc.declare_dram_parameter("bpack", [128, BPACK_COLS], f32,
                                      isOutput=False)
    out = nc.declare_dram_parameter("out", [128, OUT_COLS], f32,
                                    isOutput=True)

    xT_r = xT.rearrange("t p (k i) -> t p k i", k=KC)
    out_r = out.rearrange("p (t s) -> p t s", t=N_TILES)

    with tile.TileContext(nc) as tc:
        with (
            tc.tile_pool(name="singles", bufs=1) as singles,
            tc.tile_pool(name="xin", bufs=4) as xin,
            tc.tile_pool(name="gates", bufs=2) as gates,
            tc.tile_pool(name="wexp", bufs=2) as wexp,
            tc.tile_pool(name="acc", bufs=1) as accp,
            tc.tile_pool(name="ps_v", bufs=2, space="PSUM") as ps_v,
            tc.tile_pool(name="ps_u", bufs=2, space="PSUM") as ps_u,
            tc.tile_pool(name="ps_lg", bufs=2, space="PSUM") as ps_lg,
            tc.tile_pool(name="ps_c", bufs=2, space="PSUM") as ps_c,
        ):
            # PE warm-up spin, emitted FIRST: the HAM clock gate keeps the
            # PE at 1.2 GHz until ~3.4us of sustained activity, so a
            # dependency-free matmul spin flips it to 2.4 GHz while the
            # preamble DMAs land.
            warm_sb = singles.tile([128, 512], f16)
            nc.vector.memset(warm_sb, 0.0)
            warm_ps = ps_lg.tile([128, 512], f32, tag="lg")
            for _ in range(14):
                nc.tensor.matmul(warm_ps[:64, :], lhsT=warm_sb[:, 0:64],
                                 rhs=warm_sb, start=True, stop=True)

            # ---- preamble: interleaved weight/x DMA ladder so the first
            # V matmul is gated on ~768KB, not ~2.6MB.
            wp = singles.tile([128, WPACK_COLS], f16)
            xt0 = xin.tile([128, KC, TILE], f16, tag="xt")
            nc.sync.dma_start(out=wp[:, 0:W_U_OFF], in_=wpack[:, 0:W_U_OFF])
            nc.sync.dma_start(out=xt0[:, 0:4, :], in_=xT_r[0, :, 0:4, :])
            nc.sync.dma_start(out=wp[:, W_U_OFF:], in_=wpack[:, W_U_OFF:])
            nc.sync.dma_start(out=xt0[:, 4:8, :], in_=xT_r[0, :, 4:8, :])
            bias_sb = singles.tile([128, BPACK_COLS], f32)
            nc.sync.dma_start(out=bias_sb, in_=bpack[:, :])
            xt1 = xin.tile([128, KC, TILE], f16, tag="xt")
            nc.sync.dma_start(out=xt1, in_=xT_r[1])

            pv = wp[:, W_V_OFF:W_U_OFF].rearrange("p (k a) -> p k a", k=KC)
            pu = wp[:, W_U_OFF:W_ATT_OFF].rearrange("p (k a) -> p k a", k=KC)
            watt128 = wp[:, W_ATT_OFF:W_CLS_OFF]
            pcls = wp[:, W_CLS_OFF:WPACK_COLS].rearrange(
                "p (k c) -> p k c", k=P_CLS_PE)

            # ACT-local biases (feed activation bias port)
            bact = singles.tile([128, BPACK_COLS], f32)
            nc.scalar.activation(bact, bias_sb, AF.Identity)

            acc = accp.tile([128, N_TILES, OUT_SEC], f32)

            xts = {0: xt0, 1: xt1}
            gvs = {}
            gus = {}
            g16s = {}

            def front_pe(t):
                """x DMA (t+2) + V/U gate matmuls for tile t."""
                if t + 2 < N_TILES:
                    xt = xin.tile([128, KC, TILE], f16, tag="xt")
                    nc.sync.dma_start(out=xt, in_=xT_r[t + 2])
                    xts[t + 2] = xt
                xt = xts[t]
                av = ps_v.tile([128, 512], f32, tag="av")
                au = ps_u.tile([128, 512], f32, tag="au")
                for k in range(KC):
                    nc.tensor.matmul(av[:, :TILE], lhsT=pv[:, k, :],
                                     rhs=xt[:, k, :],
                                     start=(k == 0), stop=(k == KC - 1))
                for k in range(KC):
                    nc.tensor.matmul(au[:, :TILE], lhsT=pu[:, k, :],
                                     rhs=xt[:, k, :],
                                     start=(k == 0), stop=(k == KC - 1))
                return av, au

            def back_pe(t, g16, lg, c_ps):
                """cls matmuls + logit matmul for tile t (g16 from an
                iteration ago, so the PE never waits on the gate chain)."""
                xt = xts[t]
                for k in range(P_CLS_PE):
                    nc.tensor.matmul(c_ps[0:NCLS, :TILE], lhsT=pcls[:, k, :],
                                     rhs=xt[:, k, :],
                                     start=(k == 0), stop=(k == P_CLS_PE - 1))
                nc.tensor.matmul(lg[:, :TILE], lhsT=watt128, rhs=g16,
                                 start=True, stop=True)

            def mid_act(t, av, au):
                """gv = tanh(yV + bV); gu = tanh(yU/2 + bU/2)."""
                gv = gates.tile([128, TILE], f16, tag="gv")
                nc.scalar.activation(gv, av[:, :TILE], AF.Tanh,
                                     bias=bact[:, 0:1])
                gu = gates.tile([128, TILE], f16, tag="gu")
                nc.scalar.activation(gu, au[:, :TILE], AF.Tanh,
                                     bias=bact[:, 1:2], scale=0.5)
                gvs[t], gus[t] = gv, gu

            def back_act(t, lg, c_ps):
                """w = exp(logit + b_att) on all 128 partitions; c16 copy."""
                w16 = wexp.tile([128, TILE], f16, tag="w16")
                nc.scalar.activation(w16, lg[:, :TILE], AF.Exp,
                                     bias=bact[:, 2:3])
                c16 = wexp.tile([NCLS, TILE], f16, tag="c16")
                nc.scalar.activation(c16, c_ps[0:NCLS, :TILE], AF.Identity)
                return w16, c16

            def mid_dve(t):
                """g16 = (gu + 1) * gv  (= 2 * A_V * A_U)."""
                g16 = gates.tile([128, TILE], f16, tag="g16")
                nc.vector.scalar_tensor_tensor(
                    g16, gus.pop(t), 1.0, gvs.pop(t), op0=ALU.add,
                    op1=ALU.mult)
                g16s[t] = g16

            def back_dve(t, w16, c16):
                """Weighted x-sum for chunks P_CLS_PE..7 + z/s partials."""
                xt = xts.pop(t)
                prod = wexp.tile([128, V_CHUNKS, TILE], f16, tag="prod")
                w_b, x_b = broadcast_tensor_aps(
                    w16.rearrange("p (o i) -> p o i", o=1),
                    xt[:, P_CLS_PE:KC, :])
                nc.vector.tensor_mul(prod[:, :, :], w_b, x_b)
                for k in range(V_CHUNKS):
                    nc.vector.reduce_sum(acc[:, t, k:k + 1], prod[:, k, :],
                                         axis=mybir.AxisListType.X)
                zprod = wexp.tile([NCLS, TILE], f16, tag="zprod")
                nc.vector.tensor_mul(zprod, w16[0:NCLS, :], c16)
                nc.vector.reduce_sum(acc[0:NCLS, t, V_CHUNKS:V_CHUNKS + 1],
                                     zprod, axis=mybir.AxisListType.X)
                nc.vector.reduce_sum(acc[0:1, t, V_CHUNKS + 1:V_CHUNKS + 2],
                                     w16[0:1, :], axis=mybir.AxisListType.X)

            avaus = {}
            wcs = {}
            for t in range(N_TILES + 2):
                lg = ps_lg.tile([128, 512], f32, tag="lg")
                c_ps = ps_c.tile([128, 512], f32, tag="c")
                if t >= 2:
                    # Observer matmul: a 1x1 read of the newest ACT output
                    # advances the PE's semaphore vector clock so the
                    # group-start matmuls' WAR legs are pre-observed.  Its
                    # dst is overwritten by the logit matmul's start=True.
                    if (t - 1) in gvs:
                        gvp = gvs[t - 1]
                        nc.tensor.matmul(lg[0:1, 0:1], lhsT=gvp[0:2, 0:1],
                                         rhs=gvp[0:2, 0:1], start=True,
                                         stop=True)
                    back_pe(t - 2, g16s.pop(t - 2), lg, c_ps)
                if t < N_TILES:
                    avaus[t] = front_pe(t)
                if 1 <= t <= N_TILES:
                    av, au = avaus.pop(t - 1)
                    mid_act(t - 1, av, au)
                if t >= 2:
                    wcs[t - 2] = back_act(t - 2, lg, c_ps)
                if 1 <= t <= N_TILES:
                    mid_dve(t - 1)
                if t >= 2:
                    w16, c16 = wcs.pop(t - 2)
                    back_dve(t - 2, w16, c16)
                if t - 2 == N_TILES - 3:
                    # early writeback of tiles 0..N_TILES-3: hides the bulk
                    # of the output DMA under the last two tiles' compute.
                    nc.sync.dma_start(out=out_r[:, 0:N_TILES - 2, :],
                                      in_=acc[:, 0:N_TILES - 2, :])

            # tail writeback: the last two tiles' partial columns
            nc.sync.dma_start(out=out_r[:, N_TILES - 2:N_TILES, :],
                              in_=acc[:, N_TILES - 2:N_TILES, :])

            # tail spin: dependency-free matmuls hold the HAM clock at
            # 2.4 GHz while the last tiles' exp/pool chain drains on
            # ACT/DVE and the TileContext teardown barrier completes.
            warm_ps2 = ps_v.tile([128, 512], f32, tag="av")
            for _ in range(40):
                nc.tensor.matmul(warm_ps2[:64, :], lhsT=warm_sb[:, 0:64],
                                 rhs=warm_sb, start=True, stop=True)

    nc.compile()
    return nc


def _prep_in_maps(x, W_pe, b_pe, W_V, b_V, W_U, b_U, w_att, b_att):
    f16 = np.float16
    f32 = np.float32
    f64 = np.float64

    W_pe64 = W_pe.astype(f64)
    P_V = (W_pe64 @ W_V.astype(f64)).astype(f32)     # [1024, 128]
    P_U = (W_pe64 @ W_U.astype(f64)).astype(f32)

    wpack = np.empty((128, WPACK_COLS), dtype=f16)
    wpack[:, W_V_OFF:W_U_OFF] = (
        P_V.reshape(KC, 128, ATT).transpose(1, 0, 2).reshape(128, KC * ATT))
    wpack[:, W_U_OFF:W_ATT_OFF] = (
        P_U.reshape(KC, 128, ATT).transpose(1, 0, 2).reshape(128, KC * ATT))
    # kernel computes G' = 2*A_V*A_U; 0.5*w_att undoes the factor of 2.
    wpack[:, W_ATT_OFF:W_CLS_OFF] = np.repeat(
        0.5 * w_att.astype(f32), 128, axis=1)
    P_cls = _cache["P_cls64"].astype(f32)             # [1024, 2]
    wpack[:, W_CLS_OFF:WPACK_COLS] = (
        P_cls[:P_CLS_PE * 128].reshape(P_CLS_PE, 128, NCLS)
        .transpose(1, 0, 2).reshape(128, P_CLS_PE * NCLS))

    bias_V = (b_pe.astype(f64) @ W_V.astype(f64) + b_V).astype(f32)
    bias_U = (b_pe.astype(f64) @ W_U.astype(f64) + b_U).astype(f32)
    bpack = np.empty((128, BPACK_COLS), dtype=f32)
    bpack[:, 0] = bias_V
    bpack[:, 1] = 0.5 * bias_U
    bpack[:, 2] = b_att[0]

    shared = {"wpack": wpack, "bpack": bpack}
    in_maps = []
    half = N // 2
    for c in range(N_CORES):
        bag, hi = divmod(c, 2)
        xs = x[bag, hi * half:(hi + 1) * half, :]
        xt_tiles = np.ascontiguousarray(
            xs.T.astype(f16).reshape(KC, 128, N_TILES, TILE)
            .transpose(2, 1, 0, 3).reshape(N_TILES, 128, KC * TILE))
        in_maps.append({"xT": xt_tiles, **shared})
    return in_maps


def _run(inputs, trace=False, tmpdir=None):
    _import_concourse()
    from concourse.bass_utils import run_bass_kernel_spmd

    if "nc" not in _cache:
        _cache["nc"] = _build_graph()
    nc = _cache["nc"]

    W_pe64 = np.asarray(inputs["W_pe"], dtype=np.float64)
    _cache["P_cls64"] = W_pe64 @ np.asarray(inputs["W_cls"], np.float64)

    in_maps = _prep_in_maps(
        inputs["x"], inputs["W_pe"], inputs["b_pe"], inputs["W_V"],
        inputs["b_V"], inputs["W_U"], inputs["b_U"], inputs["w_att"],
        inputs["b_att"])

    res = run_bass_kernel_spmd(
        nc, in_maps, core_ids=list(range(N_CORES)),
        trace=trace, tmpdir=tmpdir)

    P_cls64 = _cache["P_cls64"]                       # [1024, 2]
    b_pe = np.asarray(inputs["b_pe"], np.float64)
    W_cls = np.asarray(inputs["W_cls"], np.float64)
    b_cls = np.asarray(inputs["b_cls"], np.float64)
    cls_const = b_pe @ W_cls + b_cls
    P_cls_dve = P_cls64[P_CLS_PE * 128:]              # [V_CHUNKS*128, 2]

    logits = np.zeros((B, NCLS), dtype=np.float32)
    for bag in range(B):
        z = np.zeros(NCLS, dtype=np.float64)
        s = 0.0
        for hi in range(2):
            o = res.results[2 * bag + hi]["out"].astype(np.float64)
            o = o.reshape(128, N_TILES, OUT_SEC)
            xbar = o[:, :, 0:V_CHUNKS].sum(axis=1)    # [128, V_CHUNKS]
            # feature index = (P_CLS_PE + k)*128 + p
            z += xbar.T.reshape(-1) @ P_cls_dve
            z += o[0:NCLS, :, V_CHUNKS].sum(axis=1)
            s += o[0, :, V_CHUNKS + 1].sum()
        logits[bag] = (z / s + cls_const).astype(np.float32)
    return logits, res


def kernel(**inputs):
    inputs = {k: np.asarray(v) for k, v in inputs.items()}
    logits, _ = _run(inputs, trace=False)
    return logits


# revision 4
# speedup vs baseline: 1.9199x; 1.4642x over previous
"""ABMIL attention-pooling kernel for 8 TRN2 NeuronCores (Bass/Tile).

Reference computation (per bag b of B=4, N=20000 instances, 1024 feats):
    h   = x @ W_pe + b_pe                    [N, 512]
    A_V = tanh(h @ W_V + b_V)                [N, 128]
    A_U = sigmoid(h @ W_U + b_U)             [N, 128]
    a   = (A_V * A_U) @ w_att + b_att        [N, 1]
    A   = softmax(a, axis=0)
    M   = sum(A * h, axis=0)                 [512]
    out = M @ W_cls + b_cls

Algebraic rewrites (all weight-only precomputation, done host-side):
  * h @ W_V == x @ (W_pe @ W_V): the attention branches never need h.
    With P_V = W_pe@W_V, P_U = W_pe@W_U (both [1024,128]) and
    bias'_V = b_pe@W_V + b_V etc., the gates are rank-256 projections of
    x directly -- the [1024x512] patch embed disappears from the device.
  * logits = (sum_n w_n h_n / s) @ W_cls + b_cls
           = (sum_n w_n (x_n @ P_cls)) / s + (b_pe@W_cls + b_cls),
    with P_cls = W_pe@W_cls [1024, 2] and w_n = exp(a_n), s = sum w_n.
    So the device only needs z = sum_n w_n c_n (c_n = x_n@P_cls, rank 2)
    and s -- never the 512-dim pooled embedding.
  * The 8 c_n matmuls (M=2, 126/128 PE columns idle) are issued as 4
    CONCURRENT column-tiled streams (tile_size 128x32): group j
    accumulates feature chunks {j, j+4} into PSUM partitions {32j,
    32j+1}.  The 4 groups stream different x chunks through different
    col-strips simultaneously, so 8 matmuls cost ~2 matmul spans.  The
    partials stay spread over 4 partition quadrants: ACT/DVE op cost
    depends only on the free dim, so one [98,500] copy / multiply /
    reduce handles all of them and the host sums 8 rows.
  * sigmoid(y) = (1 + tanh(y/2))/2 so tanh/exp/identity are the only ACT
    functions -> one ACT table set, no table swaps.  0.5 factors fold
    into w_att (host) and the U-branch bias.
  * w_att is replicated to 128 columns host-side so the logit matmul
    broadcasts the logit onto all 128 partitions (w16 must align with
    the c partials in all 4 quadrants).

fp8 was evaluated and rejected: quantizing the P_* weights creates an
instance-correlated attention tilt that does NOT average out in the pool
(measured 8.7e-2 rel err vs the 2e-2 gate), and DoubleRow requires both
operands fp8.  f16 everywhere measures ~8.5e-4.

A DVE weighted-x-sum variant (xbar path) was measured at 7.35us/tile of
DVE time (TENSOR_REDUCE is 1x mode, ~780ns per [128,500] reduce) and
DVE-bound 161us total; this all-PE version keeps DVE at ~2.8us/tile.

Sharding: core c -> bag c//2, instance half c%2 (10000 instances each);
the host sums the two partials per bag and applies the constant.

Pipeline (software-pipelined 3 deep, per python iteration t):
    PE : cls(t-2) 8MM-in-2-spans + logit(t-2) + V(t) 8MM + U(t) 8MM
    ACT: gv(t-1), gu(t-1) tanh; w16(t-2) exp; c16(t-2) copy
    DVE: g16(t-1) gate product; zprod(t-2); zred(t-2); sred(t-2)
PE is the bottleneck engine (~5.3us/iter with the measured 262ns/MM
full-weight-reload pacing); ACT ~3.2us, DVE ~2.8us.
"""

import os
import sys

import numpy as np

# Shapes for this problem (hardcoded per the task contract).
B = 4
N = 20000
IN_DIM = 1024
EMB = 512
ATT = 128
NCLS = 2
N_CORES = 8
N_SHARD = (B * N) // N_CORES  # 10000 instances per core
TILE = 500                    # instances per device tile
N_TILES = N_SHARD // TILE     # 20
KC = IN_DIM // 128            # 8 feature chunks
N_GRP = 4                     # concurrent cls col-tile groups
K_PER_GRP = KC // N_GRP       # feature chunks accumulated per group

# packed weight layout (f16, [128, WPACK_COLS]):
#   [0:1024)        P_V     as [ki, k*128+a]
#   [1024:2048)     P_U     as [ki, k*128+a]
#   [2048:2176)     watt128 (0.5*w_att replicated to 128 cols)
#   [2176:2192)     P_cls   as [ki, k*2+c]
W_V_OFF = 0
W_U_OFF = 1024
W_ATT_OFF = 2048
W_CLS_OFF = 2176
WPACK_COLS = W_CLS_OFF + KC * NCLS

# packed bias layout (f32, [128, 3]): [0] bias'_V; [1] 0.5*bias'_U;
# [2] b_att (broadcast all rows)
BPACK_COLS = 3

# out layout (f32, [128, N_TILES, 2]): [:, t, 0] = z partials on rows
# {32j, 32j+1}; [0, t, 1] = s partial.
OUT_SEC = 2
OUT_COLS = N_TILES * OUT_SEC
ZROWS = 32 * (N_GRP - 1) + NCLS   # 98: partition rows spanned by c partials

_cache = {}


def _import_concourse():
    for p in ("/opt/trn_rl_repo", "/root/.axon_site",
              "/root/.axon_site/_ro/trn_rl_repo"):
        if os.path.isdir(p) and p not in sys.path:
            sys.path.append(p)
    import concourse.bass as bass          # noqa: F401
    import concourse.tile as tile          # noqa: F401
    from concourse import mybir            # noqa: F401
    return bass, tile, mybir


def _build_graph():
    bass, tile, mybir = _import_concourse()
    from concourse import bacc
    f16 = mybir.dt.float16
    f32 = mybir.dt.float32
    AF = mybir.ActivationFunctionType
    ALU = mybir.AluOpType

    nc = bacc.Bacc("TRN2", target_bir_lowering=False, debug=False,
                   num_devices=N_CORES)

    # x ships tile-major: [tile, ki, k*TILE+inst] so each tile is one DMA
    # with an 8KB contiguous run per partition.
    xT = nc.declare_dram_parameter("xT", [N_TILES, 128, KC * TILE], f16,
                                   isOutput=False)
    wpack = nc.declare_dram_parameter("wpack", [128, WPACK_COLS], f16,
                                      isOutput=False)
    bpack = nc.declare_dram_parameter("bpack", [128, BPACK_COLS], f32,
                                      isOutput=False)
    out = nc.declare_dram_parameter("out", [128, OUT_COLS], f32,
                                    isOutput=True)

    xT_r = xT.rearrange("t p (k i) -> t p k i", k=KC)
    out_r = out.rearrange("p (t s) -> p t s", t=N_TILES)

    with tile.TileContext(nc) as tc:
        with (
            tc.tile_pool(name="singles", bufs=1) as singles,
            tc.tile_pool(name="xin", bufs=4) as xin,
            tc.tile_pool(name="gates", bufs=2) as gates,
            tc.tile_pool(name="wexp", bufs=2) as wexp,
            tc.tile_pool(name="acc", bufs=1) as accp,
            tc.tile_pool(name="ps_v", bufs=2, space="PSUM") as ps_v,
            tc.tile_pool(name="ps_u", bufs=2, space="PSUM") as ps_u,
            tc.tile_pool(name="ps_lg", bufs=2, space="PSUM") as ps_lg,
            tc.tile_pool(name="ps_c", bufs=2, space="PSUM") as ps_c,
        ):
            # PE warm-up spin, emitted FIRST: the HAM clock gate keeps the
            # PE at 1.2 GHz until ~3.4us of sustained activity, so a
            # dependency-free matmul spin flips it to 2.4 GHz while the
            # preamble DMAs land.
            warm_sb = singles.tile([128, 512], f16)
            nc.vector.memset(warm_sb, 0.0)
            warm_ps = ps_lg.tile([128, 512], f32, tag="lg")
            for _ in range(14):
                nc.tensor.matmul(warm_ps[:64, :], lhsT=warm_sb[:, 0:64],
                                 rhs=warm_sb, start=True, stop=True)

            # ---- preamble: interleaved weight/x DMA ladder so the first
            # V matmul is gated on ~768KB, not ~2.6MB.
            wp = singles.tile([128, WPACK_COLS], f16)
            xt0 = xin.tile([128, KC, TILE], f16, tag="xt")
            nc.sync.dma_start(out=wp[:, 0:W_U_OFF], in_=wpack[:, 0:W_U_OFF])
            nc.sync.dma_start(out=xt0[:, 0:4, :], in_=xT_r[0, :, 0:4, :])
            nc.sync.dma_start(out=wp[:, W_U_OFF:], in_=wpack[:, W_U_OFF:])
            nc.sync.dma_start(out=xt0[:, 4:8, :], in_=xT_r[0, :, 4:8, :])
            bias_sb = singles.tile([128, BPACK_COLS], f32)
            nc.sync.dma_start(out=bias_sb, in_=bpack[:, :])
            xt1 = xin.tile([128, KC, TILE], f16, tag="xt")
            nc.sync.dma_start(out=xt1, in_=xT_r[1])

            pv = wp[:, W_V_OFF:W_U_OFF].rearrange("p (k a) -> p k a", k=KC)
            pu = wp[:, W_U_OFF:W_ATT_OFF].rearrange("p (k a) -> p k a", k=KC)
            watt128 = wp[:, W_ATT_OFF:W_CLS_OFF]
            pcls = wp[:, W_CLS_OFF:WPACK_COLS].rearrange(
                "p (k c) -> p k c", k=KC)

            # ACT-local biases (feed activation bias port)
            bact = singles.tile([128, BPACK_COLS], f32)
            nc.scalar.activation(bact, bias_sb, AF.Identity)

            acc = accp.tile([128, N_TILES, OUT_SEC], f32)

            xts = {0: xt0, 1: xt1}
            gvs = {}
            gus = {}
            g16s = {}

            def front_pe(t):
                """x DMA (t+2) + V/U gate matmuls for tile t."""
                if t + 2 < N_TILES:
                    xt = xin.tile([128, KC, TILE], f16, tag="xt")
                    nc.sync.dma_start(out=xt, in_=xT_r[t + 2])
                    xts[t + 2] = xt
                xt = xts[t]
                av = ps_v.tile([128, 512], f32, tag="av")
                au = ps_u.tile([128, 512], f32, tag="au")
                for k in range(KC):
                    nc.tensor.matmul(av[:, :TILE], lhsT=pv[:, k, :],
                                     rhs=xt[:, k, :],
                                     start=(k == 0), stop=(k == KC - 1))
                for k in range(KC):
                    nc.tensor.matmul(au[:, :TILE], lhsT=pu[:, k, :],
                                     rhs=xt[:, k, :],
                                     start=(k == 0), stop=(k == KC - 1))
                return av, au

            def back_pe(t, g16, lg, c_ps):
                """cls matmuls (4 concurrent col-tile groups, K_PER_GRP
                sequential accumulations each) + logit matmul for tile t
                (g16 from an iteration ago, so the PE never waits)."""
                xt = xts[t]
                for r in range(K_PER_GRP):
                    for j in range(N_GRP):
                        k = j + N_GRP * r
                        p0 = 32 * j
                        nc.tensor.matmul(
                            c_ps[p0:p0 + NCLS, :TILE], lhsT=pcls[:, k, :],
                            rhs=xt[:, k, :], tile_position=(0, p0),
                            start=(r == 0), stop=(r == K_PER_GRP - 1))
                nc.tensor.matmul(lg[:, :TILE], lhsT=watt128, rhs=g16,
                                 start=True, stop=True)

            def mid_act(t, av, au):
                """gv = tanh(yV + bV); gu = tanh(yU/2 + bU/2)."""
                gv = gates.tile([128, TILE], f16, tag="gv")
                nc.scalar.activation(gv, av[:, :TILE], AF.Tanh,
                                     bias=bact[:, 0:1])
                gu = gates.tile([128, TILE], f16, tag="gu")
                nc.scalar.activation(gu, au[:, :TILE], AF.Tanh,
                                     bias=bact[:, 1:2], scale=0.5)
                gvs[t], gus[t] = gv, gu

            def back_act(t, lg, c_ps):
                """w = exp(logit + b_att) on all 128 partitions; c16 copy
                of the 4 quadrant partials in one op (cost is FD-bound)."""
                w16 = wexp.tile([128, TILE], f16, tag="w16")
                nc.scalar.activation(w16, lg[:, :TILE], AF.Exp,
                                     bias=bact[:, 2:3])
                c16 = wexp.tile([128, TILE], f16, tag="c16")
                nc.scalar.activation(c16[0:ZROWS, :], c_ps[0:ZROWS, :TILE],
                                     AF.Identity)
                return w16, c16

            def mid_dve(t):
                """g16 = (gu + 1) * gv  (= 2 * A_V * A_U)."""
                g16 = gates.tile([128, TILE], f16, tag="g16")
                nc.vector.scalar_tensor_tensor(
                    g16, gus.pop(t), 1.0, gvs.pop(t), op0=ALU.add,
                    op1=ALU.mult)
                g16s[t] = g16

            def back_dve(t, w16, c16):
                """z/s partials: one FD-bound multiply+reduce covers all
                4 quadrants; garbage rows in between are never read."""
                del xts[t]
                zprod = wexp.tile([128, TILE], f16, tag="zprod")
                nc.vector.tensor_mul(zprod[0:ZROWS, :], w16[0:ZROWS, :],
                                     c16[0:ZROWS, :])
                nc.vector.reduce_sum(acc[0:ZROWS, t, 0:1], zprod[0:ZROWS, :],
                                     axis=mybir.AxisListType.X)
                nc.vector.reduce_sum(acc[0:1, t, 1:2], w16[0:1, :],
                                     axis=mybir.AxisListType.X)

            avaus = {}
            wcs = {}
            for t in range(N_TILES + 2):
                lg = ps_lg.tile([128, 512], f32, tag="lg")
                c_ps = ps_c.tile([128, 512], f32, tag="c")
                if t >= 2:
                    back_pe(t - 2, g16s.pop(t - 2), lg, c_ps)
                if t < N_TILES:
                    avaus[t] = front_pe(t)
                if 1 <= t <= N_TILES:
                    av, au = avaus.pop(t - 1)
                    mid_act(t - 1, av, au)
                if t >= 2:
                    wcs[t - 2] = back_act(t - 2, lg, c_ps)
                if 1 <= t <= N_TILES:
                    mid_dve(t - 1)
                if t >= 2:
                    w16, c16 = wcs.pop(t - 2)
                    back_dve(t - 2, w16, c16)
                if t - 2 == N_TILES - 3:
                    # early writeback of tiles 0..N_TILES-3: hides the bulk
                    # of the output DMA under the last two tiles' compute.
                    nc.sync.dma_start(out=out_r[:, 0:N_TILES - 2, :],
                                      in_=acc[:, 0:N_TILES - 2, :])

            # tail writeback: the last two tiles' partial columns
            nc.sync.dma_start(out=out_r[:, N_TILES - 2:N_TILES, :],
                              in_=acc[:, N_TILES - 2:N_TILES, :])

            # tail spin: dependency-free matmuls hold the HAM clock at
            # 2.4 GHz while the last tiles' exp/pool chain drains on
            # ACT/DVE and the TileContext teardown barrier completes.
            warm_ps2 = ps_v.tile([128, 512], f32, tag="av")
            for _ in range(40):
                nc.tensor.matmul(warm_ps2[:64, :], lhsT=warm_sb[:, 0:64],
                                 rhs=warm_sb, start=True, stop=True)

    nc.compile()
    return nc


def _prep_in_maps(x, W_pe, b_pe, W_V, b_V, W_U, b_U, w_att, b_att):
    f16 = np.float16
    f32 = np.float32
    f64 = np.float64

    W_pe64 = W_pe.astype(f64)
    P_V = (W_pe64 @ W_V.astype(f64)).astype(f32)     # [1024, 128]
    P_U = (W_pe64 @ W_U.astype(f64)).astype(f32)

    wpack = np.empty((128, WPACK_COLS), dtype=f16)
    wpack[:, W_V_OFF:W_U_OFF] = (
        P_V.reshape(KC, 128, ATT).transpose(1, 0, 2).reshape(128, KC * ATT))
    wpack[:, W_U_OFF:W_ATT_OFF] = (
        P_U.reshape(KC, 128, ATT).transpose(1, 0, 2).reshape(128, KC * ATT))
    # kernel computes G' = 2*A_V*A_U; 0.5*w_att undoes the factor of 2.
    wpack[:, W_ATT_OFF:W_CLS_OFF] = np.repeat(
        0.5 * w_att.astype(f32), 128, axis=1)
    P_cls = _cache["P_cls64"].astype(f32)             # [1024, 2]
    wpack[:, W_CLS_OFF:WPACK_COLS] = (
        P_cls.reshape(KC, 128, NCLS).transpose(1, 0, 2)
        .reshape(128, KC * NCLS))

    bias_V = (b_pe.astype(f64) @ W_V.astype(f64) + b_V).astype(f32)
    bias_U = (b_pe.astype(f64) @ W_U.astype(f64) + b_U).astype(f32)
    bpack = np.empty((128, BPACK_COLS), dtype=f32)
    bpack[:, 0] = bias_V
    bpack[:, 1] = 0.5 * bias_U
    bpack[:, 2] = b_att[0]

    shared = {"wpack": wpack, "bpack": bpack}
    in_maps = []
    half = N // 2
    for c in range(N_CORES):
        bag, hi = divmod(c, 2)
        xs = x[bag, hi * half:(hi + 1) * half, :]
        xt_tiles = np.ascontiguousarray(
            xs.T.astype(f16).reshape(KC, 128, N_TILES, TILE)
            .transpose(2, 1, 0, 3).reshape(N_TILES, 128, KC * TILE))
        in_maps.append({"xT": xt_tiles, **shared})
    return in_maps


def _run(inputs, trace=False, tmpdir=None):
    _import_concourse()
    from concourse.bass_utils import run_bass_kernel_spmd

    if "nc" not in _cache:
        _cache["nc"] = _build_graph()
    nc = _cache["nc"]

    W_pe64 = np.asarray(inputs["W_pe"], dtype=np.float64)
    _cache["P_cls64"] = W_pe64 @ np.asarray(inputs["W_cls"], np.float64)

    in_maps = _prep_in_maps(
        inputs["x"], inputs["W_pe"], inputs["b_pe"], inputs["W_V"],
        inputs["b_V"], inputs["W_U"], inputs["b_U"], inputs["w_att"],
        inputs["b_att"])

    res = run_bass_kernel_spmd(
        nc, in_maps, core_ids=list(range(N_CORES)),
        trace=trace, tmpdir=tmpdir)

    b_pe = np.asarray(inputs["b_pe"], np.float64)
    W_cls = np.asarray(inputs["W_cls"], np.float64)
    b_cls = np.asarray(inputs["b_cls"], np.float64)
    cls_const = b_pe @ W_cls + b_cls

    logits = np.zeros((B, NCLS), dtype=np.float32)
    for bag in range(B):
        z = np.zeros(NCLS, dtype=np.float64)
        s = 0.0
        for hi in range(2):
            o = res.results[2 * bag + hi]["out"].astype(np.float64)
            o = o.reshape(128, N_TILES, OUT_SEC)
            for j in range(N_GRP):
                z += o[32 * j:32 * j + NCLS, :, 0].sum(axis=1)
            s += o[0, :, 1].sum()
        logits[bag] = (z / s + cls_const).astype(np.float32)
    return logits, res


def kernel(**inputs):
    inputs = {k: np.asarray(v) for k, v in inputs.items()}
    logits, _ = _run(inputs, trace=False)
    return logits
